# revision 1
# baseline (speedup 1.0000x reference)
"""Trainium2 Bass kernel for the audio/visual contrastive loss.

Strategy: K-parallel sharding of the big matmuls. The embedding matmul
E = [A;V] @ [W_a;W_v] contracts K (visual K=69120 dominates). Each of the
8 cores gets a 1/8 slice of the contraction dim (both the activations'
feature columns and the matching weight rows), computes a partial
E.T (512d x 1024samples) with fp32r matmuls (PE transposes bring X into
k-major layout), the partials are AllReduced (bf16 payload), and every
core computes the small loss tail (norms, Gram, exp/log/mean) redundantly
with the raw Gram overlapping the norm-recip chain.

Per-core HBM traffic is ~36 MB (vs ~160 MB for batch-parallel sharding,
which would replicate the 141 MB W_v on every core).
"""

import sys

sys.path.insert(0, "/opt/trn_rl_repo")

import numpy as np

import concourse.bass as bass
import concourse.mybir as mybir
import concourse.tile as tile
from concourse import bacc, bass_utils
from concourse.bass import ts
from concourse.masks import make_identity

N_CORES = 8
B = 256          # batch
S = 2 * B        # samples per modality after the pair-concat
D = 512          # embedding dim
KV_TOT = 3 * 5 * 48 * 96   # 69120 visual features (lower half)
KV = KV_TOT // N_CORES     # 8640 per core
KA_TOT = 1280
KA = KA_TOT // N_CORES     # 160 per core
F32 = mybir.dt.float32
F32R = mybir.dt.float32r
BF16 = mybir.dt.bfloat16
AF = mybir.ActivationFunctionType

_CACHE = {}


def build():
    nc = bacc.Bacc("TRN2", target_bir_lowering=False, debug=False,
                   num_devices=N_CORES)

    xv_d = nc.dram_tensor("xv", [S, KV], F32R, kind="ExternalInput")
    wv_d = nc.dram_tensor("wv", [KV, D], F32R, kind="ExternalInput")
    xa_d = nc.dram_tensor("xa", [S, KA], F32, kind="ExternalInput")
    wa_d = nc.dram_tensor("wa", [KA, D], F32, kind="ExternalInput")
    loss_d = nc.dram_tensor("loss", [1, 1], F32, kind="ExternalOutput")

    # visual k tiles: 67 x 128 + 1 x 64
    kts = [128] * (KV // 128) + ([KV % 128] if KV % 128 else [])
    NKT = len(kts)

    with tile.TileContext(nc) as tc:
        with tc.tile_pool(name="const", bufs=1) as constp, \
             tc.tile_pool(name="emb", bufs=1) as embp:
            ident = constp.tile([128, 128], F32)
            make_identity(nc, ident[:])
            ident_r = constp.tile([128, 128], F32R)
            nc.vector.tensor_copy(ident_r[:], ident[:])
            ones_f = constp.tile([128, 1], F32)
            nc.vector.memset(ones_f[:], 1.0)
            ones_r = constp.tile([128, 1], F32R)
            nc.vector.tensor_copy(ones_r[:], ones_f[:])
            ones_row_f = constp.tile([1, 128], F32)
            nc.vector.memset(ones_row_f[:], 1.0)
            ones_row_r = constp.tile([1, 128], F32R)
            nc.vector.tensor_copy(ones_row_r[:], ones_row_f[:])
            # preload ACT function tables during the k-loop
            warm = constp.tile([1, 4], F32)
            nc.vector.memset(warm[:], 1.0)
            for fn in (AF.Exp, AF.Sqrt, AF.Ln):
                nc.scalar.activation(warm[:], warm[:], fn)

            # E.T partial, (512 d, 1024 s): audio cols 0:512, visual 512:1024
            # bf16 so the AllReduce moves half the bytes.
            e_sb = [embp.tile([128, 2 * S], BF16, tag=f"e{d}", name=f"e_sb{d}")
                    for d in range(4)]

            xv_r = xv_d.ap().rearrange("(a p) k -> p a k", p=128)
            xa_r = xa_d.ap().rearrange("(a p) k -> p a k", p=128)

            # -- Phase A/B: partial E.T (audio first, then visual k-loop) --
            e_sb = [embp.tile([128, 2 * S], BF16, tag=f"e{d}", name=f"e_sb{d}")
                    for d in range(4)]

            with tc.tile_pool(name="xin", bufs=6) as xinp, \
                 tc.tile_pool(name="win", bufs=8) as winp, \
                 tc.tile_pool(name="wr", bufs=4) as wrp, \
                 tc.tile_pool(name="xt", bufs=5) as xtp, \
                 tc.tile_pool(name="pacc", bufs=1, space="PSUM") as paccp, \
                 tc.tile_pool(name="ptr", bufs=3, space="PSUM") as ptrp, \
                 tc.tile_pool(name="pa", bufs=1, space="PSUM") as pap:
                psum_v = [paccp.tile([128, S], F32, tag=f"pv{d}",
                                     name=f"psum_v{d}")
                          for d in range(4)]

                # ---- audio partial (cheap, fills the DMA warmup bubble) ----
                x_a = xinp.tile([128, 4, KA], F32, tag="xa")
                nc.sync.dma_start(out=x_a[:], in_=xa_r[:])
                wa_sb = winp.tile([128, D], F32, tag="wa0")
                nc.sync.dma_start(out=wa_sb[0:128, :], in_=wa_d.ap()[0:128, :])
                wa1_sb = winp.tile([32, D], F32, tag="wa1")
                nc.sync.dma_start(out=wa1_sb[:], in_=wa_d.ap()[128:KA, :])
                war0 = wrp.tile([128, D], F32R, tag="war0")
                nc.scalar.copy(war0[:], wa_sb[:])
                war1 = wrp.tile([32, D], F32R, tag="war1")
                nc.scalar.copy(war1[:], wa1_sb[:])

                pst0 = ptrp.tile([128, 512], F32, tag="pst", name="pst")
                for j in range(4):
                    nc.tensor.transpose(pst0[0:128, ts(j, 128)],
                                        x_a[:, j, 0:128], ident[:])
                xta0 = xtp.tile([128, S], F32R, tag="xta0")
                nc.vector.tensor_copy(xta0[:], pst0[:])
                pst1 = ptrp.tile([128, 512], F32, tag="pst", name="pst")
                for j in range(4):
                    nc.tensor.transpose(pst1[0:32, ts(j, 128)],
                                        x_a[:, j, 128:KA], ident[:])
                xta1 = xtp.tile([32, S], F32R, tag="xta1")
                nc.vector.tensor_copy(xta1[:], pst1[0:32, :])

                for d in range(4):
                    pa_d = pap.tile([128, S], F32)
                    nc.tensor.matmul(pa_d[:], war0[:, ts(d, 128)], xta0[:],
                                     start=True, stop=False)
                    nc.tensor.matmul(pa_d[:], war1[:, ts(d, 128)], xta1[:],
                                     start=False, stop=True)
                    nc.vector.tensor_copy(e_sb[d][:, 0:S], pa_d[:])

                # ---- visual k-loop ----
                k0 = 0
                for kt, kw in enumerate(kts):
                    x_kt = xinp.tile([128, 4, 128], F32R)
                    nc.sync.dma_start(out=x_kt[:, :, 0:kw],
                                      in_=xv_r[:, :, k0:k0 + kw])
                    w_r = winp.tile([128, D], F32R)
                    nc.sync.dma_start(out=w_r[0:kw, :],
                                      in_=wv_d.ap()[k0:k0 + kw, :])

                    pst = ptrp.tile([128, 512], F32R, tag="pst", name="pst")
                    for j in range(4):
                        nc.tensor.transpose(pst[0:kw, ts(j, 128)],
                                            x_kt[:, j, 0:kw], ident_r[:])
                    xt = xtp.tile([128, S], F32R, tag="xt", name="xt")
                    nc.vector.tensor_copy(xt[0:kw, :], pst[0:kw, :])

                    for d in range(4):
                        nc.tensor.matmul(psum_v[d][:],
                                         w_r[0:kw, ts(d, 128)],
                                         xt[0:kw, :],
                                         start=(kt == 0), stop=(kt == NKT - 1))
                    k0 += kw

                for d in range(4):
                    nc.vector.tensor_copy(e_sb[d][:, S:2 * S], psum_v[d][:])
                # re-warm ACT tables during the AllReduce window
                for fn in (AF.Ln, AF.Exp, AF.Sqrt):
                    nc.scalar.activation(warm[:], warm[:], fn)

            # ---------------- Phase C: AllReduce partials (bf16) ----------
            with tc.tile_pool(name="dram", bufs=1, space="DRAM") as dramp, \
                 tc.tile_pool(name="red", bufs=1) as redp:
                in_b = dramp.tile([4 * 128, 2 * S], BF16)
                out_b = dramp.tile([4 * 128, 2 * S], BF16)
                for d in range(4):
                    nc.sync.dma_start(out=in_b[ts(d, 128), :], in_=e_sb[d][:])
                nc.gpsimd.collective_compute(
                    "AllReduce", mybir.AluOpType.add,
                    replica_groups=[list(range(N_CORES))],
                    ins=[in_b.opt()], outs=[out_b.opt()],
                )
                er = []
                for d in range(4):
                    rd = redp.tile([128, 2 * S], BF16, tag=f"r{d}",
                                   name=f"r{d}")
                    nc.sync.dma_start(out=rd[:], in_=out_b[ts(d, 128), :])
                    er.append(rd)

                # ---------------- Phase D: loss tail ----------------
                with tc.tile_pool(name="tail", bufs=1) as tp, \
                     tc.tile_pool(name="ptail", bufs=2, space="PSUM") as ptp, \
                     tc.tile_pool(name="prow", bufs=1, space="PSUM") as prp:
                    # f32r copies of the reduced E.T for the raw Gram work
                    er_r = [tp.tile([128, 2 * S], F32R, tag=f"err{d}",
                                    name=f"er_r{d}")
                            for d in range(4)]
                    sq = [tp.tile([128, 2 * S], F32R, tag=f"sq{d}",
                                  name=f"sq{d}")
                          for d in range(4)]
                    for d in range(4):
                        nc.vector.tensor_copy(er_r[d][:], er[d][:])
                        nc.vector.tensor_mul(sq[d][:], er[d][:], er[d][:])

                    # raw Gram block a x v (starts while norms chain runs)
                    psm = [ptp.tile([128, 512], F32, tag="psm",
                                    name=f"psm{at}")
                           for at in range(4)]
                    for at in range(4):
                        for d in range(4):
                            nc.tensor.matmul(psm[at][:],
                                             er_r[d][:, ts(at, 128)],
                                             er_r[d][:, S:2 * S],
                                             start=(d == 0), stop=(d == 3))

                    # raw diag products (6 pairs x 256 cols)
                    pairs = [(0, 512), (0, 768), (256, 512), (256, 768),
                             (0, 256), (512, 768)]
                    tprod = [tp.tile([128, 6 * 256], F32R, tag=f"tp{d}",
                                     name=f"tprod{d}")
                             for d in range(4)]
                    for d in range(4):
                        for i, (c1, c2) in enumerate(pairs):
                            nc.vector.tensor_mul(
                                tprod[d][:, ts(i, 256)],
                                er_r[d][:, c1:c1 + 256],
                                er_r[d][:, c2:c2 + 256])
                    traw = prp.tile([1, 6 * 256], F32, name="traw")
                    for g in range(3):
                        for d in range(4):
                            nc.tensor.matmul(traw[:, ts(g, 512)], ones_r[:],
                                             tprod[d][:, ts(g, 512)],
                                             start=(d == 0), stop=(d == 3))

                    # norms chain: sq -> norms2 -> sqrt -> 1/norm
                    norm_row = tp.tile([1, 2 * S], F32)
                    for h in range(2):
                        psh = prp.tile([1, 512], F32, tag="row", name="psh", bufs=2)
                        for d in range(4):
                            nc.tensor.matmul(psh[:], ones_r[:],
                                             sq[d][:, ts(h, 512)],
                                             start=(d == 0), stop=(d == 3))
                        nc.scalar.activation(norm_row[:, ts(h, 512)], psh[:],
                                             AF.Sqrt)
                    rn = tp.tile([1, 2 * S], F32)
                    nc.vector.reciprocal(rn[:], norm_row[:])

                    # rn as columns (4 PE transposes) for the exp scale
                    rn_col = tp.tile([128, 4], F32)
                    for at in range(4):
                        prc = prp.tile([128, 1], F32, tag="row", name="prc",
                                       bufs=2)
                        nc.tensor.transpose(prc[:], rn[0:1, ts(at, 128)],
                                            ident[0:1, 0:1])
                        nc.vector.tensor_copy(rn_col[:, at:at + 1], prc[:])

                    # broadcast visual 1/norm along partitions via K=1 matmul
                    rnv_r = tp.tile([1, 512], F32R)
                    nc.vector.tensor_copy(rnv_r[:], rn[0:1, S:2 * S])
                    rnv_bc = tp.tile([128, 512], F32)
                    psb = prp.tile([128, 512], F32, name="psb")
                    nc.tensor.matmul(psb[:], ones_row_r[:], rnv_r[:],
                                     start=True, stop=True)
                    nc.vector.tensor_copy(rnv_bc[:], psb[:])

                    # denominator: rowsum of exp(M * rn_i * rn_j)
                    denp = tp.tile([128, 4], F32)
                    junk = tp.tile([128, 512], F32, tag="junk")
                    mn = tp.tile([128, 512], F32, tag="mn")
                    for at in range(4):
                        nc.vector.tensor_mul(mn[:], psm[at][:], rnv_bc[:])
                        nc.scalar.activation(junk[:], mn[:], AF.Exp,
                                             scale=rn_col[:, at:at + 1],
                                             accum_out=denp[:, at:at + 1])
                    den2 = tp.tile([128, 2], F32)
                    for j in range(2):
                        nc.vector.tensor_add(den2[:, j:j + 1],
                                             denp[:, j:j + 1],
                                             denp[:, j + 2:j + 3])

                    # numerator: exp of scaled diag terms
                    rnp = tp.tile([1, 6 * 256], F32)
                    for i, (c1, c2) in enumerate(pairs):
                        nc.vector.tensor_mul(rnp[:, ts(i, 256)],
                                             rn[0:1, c1:c1 + 256],
                                             rn[0:1, c2:c2 + 256])
                    that = tp.tile([1, 6 * 256], F32)
                    nc.vector.tensor_mul(that[:], traw[:], rnp[:])
                    exp_t = tp.tile([1, 6 * 256], F32)
                    nc.scalar.activation(exp_t[:], that[:], AF.Exp)
                    num = tp.tile([1, 256], F32)
                    nc.vector.tensor_add(num[:], exp_t[:, 0:256],
                                         exp_t[:, 256:512])
                    for i in range(2, 6):
                        nc.vector.tensor_add(num[:], num[:],
                                             exp_t[:, ts(i, 256)])

                    # denominator columns -> row via PE transpose
                    den_row = tp.tile([1, 256], F32)
                    for j in range(2):
                        pdr = prp.tile([1, 128], F32, tag="row", name="pdr", bufs=2)
                        nc.tensor.transpose(pdr[:], den2[:, j:j + 1], ident[:])
                        nc.vector.tensor_copy(den_row[:, ts(j, 128)], pdr[:])

                    rden = tp.tile([1, 256], F32)
                    nc.vector.reciprocal(rden[:], den_row[:])
                    ratio = tp.tile([1, 256], F32)
                    nc.vector.tensor_mul(ratio[:], num[:], rden[:])
                    logr = tp.tile([1, 256], F32)
                    nc.scalar.activation(logr[:], ratio[:], AF.Ln)
                    lsum = tp.tile([1, 1], F32)
                    nc.vector.reduce_sum(lsum[:], logr[:],
                                         axis=mybir.AxisListType.X)
                    loss_sb = tp.tile([1, 1], F32)
                    nc.scalar.activation(loss_sb[:], lsum[:], AF.Copy,
                                         scale=float(-1.0 / B))
                    nc.sync.dma_start(out=loss_d.ap(), in_=loss_sb[:])

    nc.compile()
    return nc


def _get_nc():
    if "nc" not in _CACHE:
        _CACHE["nc"] = build()
    return _CACHE["nc"]


def _shard_inputs(a_1, v_1, a_2, v_2, W_a, W_v):
    # audio: (2b,1,80,16) -> (512, 1280)
    A = np.concatenate([a_1, a_2], axis=0).reshape(S, KA_TOT)
    # visual: (2b,3,5,96,96), keep lower half rows, flatten in native
    # (c,t,r,w) order; W_v rows permuted to match ((t,c)->(c,t) blocks).
    V = np.concatenate([v_1, v_2], axis=0)
    V = V.reshape(S, 15, 96, 96)[:, :, 48:, :].reshape(S, KV_TOT)
    Wvp = np.ascontiguousarray(
        W_v.reshape(5, 3, 48 * 96, D).transpose(1, 0, 2, 3)
    ).reshape(KV_TOT, D)

    in_maps = []
    for c in range(N_CORES):
        in_maps.append({
            "xv": np.ascontiguousarray(V[:, c * KV:(c + 1) * KV]),
            "wv": np.ascontiguousarray(Wvp[c * KV:(c + 1) * KV, :]),
            "xa": np.ascontiguousarray(A[:, c * KA:(c + 1) * KA]),
            "wa": np.ascontiguousarray(W_a[c * KA:(c + 1) * KA, :]),
        })
    return in_maps


def kernel(a_1, v_1, a_2, v_2, W_a, W_v):
    nc = _get_nc()
    in_maps = _shard_inputs(np.asarray(a_1, np.float32),
                            np.asarray(v_1, np.float32),
                            np.asarray(a_2, np.float32),
                            np.asarray(v_2, np.float32),
                            np.asarray(W_a, np.float32),
                            np.asarray(W_v, np.float32))
    res = bass_utils.run_bass_kernel_spmd(nc, in_maps,
                                          core_ids=list(range(N_CORES)))
    return np.asarray(res.results[0]["loss"], np.float32).reshape(())



# revision 12
# speedup vs baseline: 2.1238x; 2.1238x over previous
"""Trainium2 Bass kernel for the audio/visual contrastive loss.

Strategy: K-parallel sharding of the visual matmul in fp8.

- Host casts inputs to fp8-e4m3 (W_v pre-scaled x256, W_a x32 -- any
  per-matrix scale cancels in the L2 normalization) and pre-transposes the
  activations to k-major, so the device does no PE transposes.
- Each core contracts a 8640-wide K slice of the visual matmul with
  fp8 DoubleRow matmuls (two 128-deep k-planes per instruction).
- The audio embedding (K=1280, tiny) is computed fully on every core, so
  the cross-core reduction only carries the visual partial E.T.
- Reduction: bf16 ReduceScatter (each core gets a 64-sample chunk of the
  reduced visual E.T), local L2-normalize of that chunk (scaled x16 for
  fp8 range), then an fp8 AllGather of the normalized embeddings.
- Tail (redundant on every core): fp8 DoubleRow Gram blocks, exp with
  row-accumulate for the denominator, diagonal extraction via
  identity-mask + row-reduce for the numerator, log/mean in column space.
"""

import sys

sys.path.insert(0, "/opt/trn_rl_repo")

import ml_dtypes
import numpy as np

import concourse.bass as bass
import concourse.mybir as mybir
import concourse.tile as tile
from concourse import bacc, bass_utils
from concourse.bass import ts
from concourse.masks import make_identity

N_CORES = 8
B = 256          # batch
S = 2 * B        # samples per modality after the pair-concat
D = 512          # embedding dim
KV_TOT = 3 * 5 * 48 * 96   # 69120 visual features (lower half)
KV = KV_TOT // N_CORES     # 8640 per core
KP = 8704                  # padded to 34 * 256
NKT = KP // 256            # 34 double-k-tiles
KA = 1280                  # audio features (not sharded)
NKA = KA // 256            # 5 double-k-tiles
F32 = mybir.dt.float32
F32R = mybir.dt.float32r
BF16 = mybir.dt.bfloat16
F8 = mybir.dt.float8e4
NP_F8 = ml_dtypes.float8_e4m3
AF = mybir.ActivationFunctionType
DR = mybir.MatmulPerfMode.DoubleRow

SC_V = 256.0    # host scale on W_v so fp8 sees ~unit-variance values
SC_A = 32.0     # host scale on W_a
EMB_SC = 16.0   # scale on normalized embeddings for fp8; Gram gets x256
GRAM_RCP = 1.0 / (EMB_SC * EMB_SC)   # exp(scale * raw_gram)

_CACHE = {}


def build():
    nc = bacc.Bacc("TRN2", target_bir_lowering=False, debug=False,
                   num_devices=N_CORES)

    # k-major fp8 inputs, pre-packed on host for DoubleRow + big DMAs
    xv_d = nc.dram_tensor("xv", [NKT, 128, 2, S], F8, kind="ExternalInput")
    wv_d = nc.dram_tensor("wv", [2, NKT, 128, 2, 256], F8, kind="ExternalInput")
    xa_d = nc.dram_tensor("xa", [NKA, 128, 2, S], F8, kind="ExternalInput")
    wa_d = nc.dram_tensor("wa", [NKA, 128, 2, D], F8, kind="ExternalInput")
    loss_d = nc.dram_tensor("loss", [1, 1], F32, kind="ExternalOutput")

    with tile.TileContext(nc) as tc:
        with tc.tile_pool(name="const", bufs=1) as constp, \
             tc.tile_pool(name="inp", bufs=1) as inp, \
             tc.tile_pool(name="emb", bufs=1) as embp, \
             tc.tile_pool(name="dram", bufs=1, space="DRAM") as dramp, \
             tc.tile_pool(name="pbig", bufs=1, space="PSUM") as pbig, \
             tc.tile_pool(name="psmall", bufs=1, space="PSUM") as psmall:
            ident = constp.tile([128, 128], F32)
            make_identity(nc, ident[:])
            ones_f = constp.tile([128, 1], F32)
            nc.vector.memset(ones_f[:], 1.0)
            ones_r = constp.tile([128, 1], F32R)
            nc.vector.tensor_copy(ones_r[:], ones_f[:])
            ones_row_f = constp.tile([1, 128], F32)
            nc.vector.memset(ones_row_f[:], 1.0)
            ones_row_r = constp.tile([1, 128], F32R)
            nc.vector.tensor_copy(ones_row_r[:], ones_row_f[:])
            warm = constp.tile([1, 4], F32)
            nc.vector.memset(warm[:], 1.0)
            for fn in (AF.Exp, AF.Sqrt, AF.Ln):
                nc.scalar.activation(warm[:], warm[:], fn)

            # ---- input DMAs (audio first: it feeds the earliest PE work) ----
            xa_sb = inp.tile([128, NKA, 2, S], F8)
            nc.sync.dma_start(
                out=xa_sb[:],
                in_=xa_d.ap().rearrange("kt p pl c -> p kt pl c"))
            wa_sb = inp.tile([128, NKA, 2, D], F8)
            nc.sync.dma_start(
                out=wa_sb[:],
                in_=wa_d.ap().rearrange("kt p pl c -> p kt pl c"))
            xv_sb = inp.tile([128, NKT, 2, S], F8)
            NCH = 2   # chunked so PE can chase the stream
            for ch in range(NCH):
                k0, k1 = ch * NKT // NCH, (ch + 1) * NKT // NCH
                nc.sync.dma_start(
                    out=xv_sb[:, k0:k1],
                    in_=xv_d.ap()[k0:k1].rearrange("kt p pl c -> p kt pl c"))
            wv_sb = [inp.tile([128, NKT, 2, 256], F8, name=f"wv_sb{g}")
                     for g in range(2)]
            for g in range(2):
                nc.sync.dma_start(
                    out=wv_sb[g][:],
                    in_=wv_d.ap()[g].rearrange("kt p pl c -> p kt pl c"))

            # ---- audio E.T (full K on every core) ----
            psum_a = [pbig.tile([128, S], F32, tag=f"pa{d}", name=f"psum_a{d}")
                      for d in range(4)]
            for d in range(4):
                for kt in range(NKA):
                    for h in range(2):
                        nc.tensor.matmul(
                            psum_a[d][:, ts(h, 256)],
                            wa_sb[:, kt, :, ts(d, 128)],
                            xa_sb[:, kt, :, ts(h, 256)],
                            start=(kt == 0 and h == 0), stop=(kt == NKA - 1),
                            perf_mode=DR, skip_group_check=True)

            # audio norms: colsum of squares -> 16/sqrt -> broadcast -> scale
            sq_a = embp.tile([128, 4, S], F32R)
            for d in range(4):
                nc.vector.tensor_mul(sq_a[:, d, :], psum_a[d][:], psum_a[d][:])
            ps_na = psmall.tile([1, S], F32, tag="rowp", name="ps_na")
            for d in range(4):
                nc.tensor.matmul(ps_na[:], ones_r[:], sq_a[:, d, :],
                                 start=(d == 0), stop=(d == 3))
            sn_a = embp.tile([1, S], F32)
            nc.scalar.activation(sn_a[:], ps_na[:], AF.Sqrt,
                                 scale=float(1.0 / (EMB_SC * EMB_SC)))
            rn_a_f = embp.tile([1, S], F32)
            nc.vector.reciprocal(rn_a_f[:], sn_a[:])
            rn_a = embp.tile([1, S], F32R)
            nc.vector.tensor_copy(rn_a[:], rn_a_f[:])
            ps_bca = psmall.tile([128, S], F32, tag="bcp", name="ps_bca")
            nc.tensor.matmul(ps_bca[:], ones_row_r[:], rn_a[:],
                             start=True, stop=True)
            bc_a = embp.tile([128, S], F32)
            nc.vector.tensor_copy(bc_a[:], ps_bca[:])
            # ea[tt][p, pl, s] = normalized audio emb, d = 128*(2tt+pl)+p
            ea = [embp.tile([128, 2, S], F8, name=f"ea{t}") for t in range(2)]
            for d in range(4):
                nc.vector.tensor_mul(ea[d // 2][:, d % 2, :],
                                     psum_a[d][:], bc_a[:])

            # ---- visual partial E.T: d-block outer, k inner ----
            rs_in = dramp.tile([8, 4, 128, 64], BF16)
            e_sb = [embp.tile([128, S], BF16, name=f"e_sb{d}")
                    for d in range(4)]
            psum_v = [pbig.tile([128, S], F32, tag=f"pa{d}", name=f"psum_v{d}")
                      for d in range(4)]
            for d in range(4):
                g, dblk = d // 2, d % 2
                for kt in range(NKT):
                    for h in range(2):
                        nc.tensor.matmul(
                            psum_v[d][:, ts(h, 256)],
                            wv_sb[g][:, kt, :, ts(dblk, 128)],
                            xv_sb[:, kt, :, ts(h, 256)],
                            start=(kt == 0 and h == 0), stop=(kt == NKT - 1),
                            perf_mode=DR, skip_group_check=True)
                nc.vector.tensor_copy(e_sb[d][:], psum_v[d][:])
                nc.sync.dma_start(
                    out=rs_in[:, d].rearrange("j p c -> p j c"),
                    in_=e_sb[d].rearrange("p (j c) -> p j c", j=8))

            # ---- ReduceScatter: core j gets reduced E.T for samples
            # [64j, 64j+64) as [4, 128, 64] (d-tile, partition, col) ----
            rs_out = dramp.tile([4, 128, 64], BF16)
            nc.gpsimd.collective_compute(
                "ReduceScatter", mybir.AluOpType.add,
                replica_groups=[list(range(N_CORES))],
                ins=[rs_in.opt()], outs=[rs_out.opt()],
            )

            red = embp.tile([128, 4, 64], BF16)
            nc.sync.dma_start(out=red[:],
                              in_=rs_out.rearrange("t p c -> p t c"))
            redf = embp.tile([128, 4, 64], F32)
            nc.vector.tensor_copy(redf[:], red[:])
            sq_v = embp.tile([128, 256], F32R)
            nc.vector.tensor_mul(sq_v[:], redf.rearrange("p t c -> p (t c)"),
                                 redf.rearrange("p t c -> p (t c)"))
            ps_nv = psmall.tile([1, 256], F32, tag="rowp", name="ps_nv")
            nc.tensor.matmul(ps_nv[:], ones_r[:], sq_v[:],
                             start=True, stop=True)
            n2a = embp.tile([1, 64], F32)
            nc.vector.tensor_add(n2a[:], ps_nv[0:1, 0:64], ps_nv[0:1, 64:128])
            n2b = embp.tile([1, 64], F32)
            nc.vector.tensor_add(n2b[:], ps_nv[0:1, 128:192],
                                 ps_nv[0:1, 192:256])
            n2 = embp.tile([1, 64], F32)
            nc.vector.tensor_add(n2[:], n2a[:], n2b[:])
            sn_v = embp.tile([1, 64], F32)
            nc.scalar.activation(sn_v[:], n2[:], AF.Sqrt,
                                 scale=float(1.0 / (EMB_SC * EMB_SC)))
            rn_v_f = embp.tile([1, 64], F32)
            nc.vector.reciprocal(rn_v_f[:], sn_v[:])
            rn_v = embp.tile([1, 64], F32R)
            nc.vector.tensor_copy(rn_v[:], rn_v_f[:])
            ps_bcv = psmall.tile([128, 64], F32, tag="bcp", name="ps_bcv")
            nc.tensor.matmul(ps_bcv[:], ones_row_r[:], rn_v[:],
                             start=True, stop=True)
            bc_v = embp.tile([128, 64], F32)
            nc.vector.tensor_copy(bc_v[:], ps_bcv[:])
            ag_sb = embp.tile([128, 4, 64], F8)
            for t in range(4):
                nc.vector.tensor_mul(ag_sb[:, t, :], redf[:, t, :], bc_v[:])

            ag_in = dramp.tile([4, 128, 64], F8)
            nc.sync.dma_start(out=ag_in.rearrange("t p c -> p t c"),
                              in_=ag_sb[:])
            ag_out = dramp.tile([8, 4, 128, 64], F8)
            nc.gpsimd.collective_compute(
                "AllGather", mybir.AluOpType.bypass,
                replica_groups=[list(range(N_CORES))],
                ins=[ag_in.opt()], outs=[ag_out.opt()],
            )
            # ev[tt][p, pl, j, c]: normalized visual emb,
            # d = 128*(2tt+pl)+p, sample = 64j+c
            ev = [embp.tile([128, 2, 8, 64], F8, name=f"ev{t}")
                  for t in range(2)]
            for tt in range(2):
                for pl in range(2):
                    nc.sync.dma_start(
                        out=ev[tt][:, pl],
                        in_=ag_out[:, 2 * tt + pl]
                            .rearrange("j p c -> p j c"))

            # ---------------- tail: Gram, exp, loss ----------------
            with tc.tile_pool(name="tail", bufs=1) as tp:
                # av Gram: psum_av[m][i, j] = a_{128m+i} . v_j  (x256)
                psum_av = [pbig.tile([128, S], F32, tag=f"pa{m}",
                                     name=f"psum_av{m}") for m in range(4)]
                for m in range(4):
                    for tt in range(2):
                        for h in range(2):
                            nc.tensor.matmul(
                                psum_av[m][:, ts(h, 256)],
                                ea[tt][:, :, ts(m, 128)],
                                ev[tt][:, :, 4 * h:4 * h + 4, :],
                                start=(tt == 0 and h == 0), stop=(tt == 1),
                                perf_mode=DR, skip_group_check=True)
                # quadrants: [a1a2 m0, a1a2 m1, v1v2 m0, v1v2 m1]
                psum_q = psmall.tile([128, S], F32, tag="bcp", name="psum_q")
                for q in range(4):
                    m = q % 2
                    for tt in range(2):
                        if q < 2:    # a1 block m  x  a2 block m
                            lhsT = ea[tt][:, :, ts(m, 128)]
                            rhs = ea[tt][:, :, 256 + 128 * m:384 + 128 * m]
                        else:        # v1 block m  x  v2 block m
                            lhsT = ev[tt][:, :, 2 * m:2 * m + 2, :]
                            rhs = ev[tt][:, :, 4 + 2 * m:6 + 2 * m, :]
                        nc.tensor.matmul(
                            psum_q[:, ts(q, 128)], lhsT, rhs,
                            start=(q == 0 and tt == 0), stop=(tt == 1),
                            perf_mode=DR, skip_group_check=True)

                # denominator: rowsum of exp(G/256) over all 512 visual
                exp_av = [tp.tile([128, S], F32, tag=f"x{m}",
                                  name=f"exp_av{m}") for m in range(4)]
                den4 = tp.tile([128, 4], F32)
                for m in range(4):
                    nc.scalar.activation(exp_av[m][:], psum_av[m][:], AF.Exp,
                                         scale=float(GRAM_RCP),
                                         accum_out=den4[:, m:m + 1])
                den2 = tp.tile([128, 2], F32)
                for j in range(2):
                    nc.vector.tensor_add(den2[:, j:j + 1], den4[:, j:j + 1],
                                         den4[:, j + 2:j + 3])

                # numerator: diagonals of the four exp'd av blocks ...
                mk = [tp.tile([128, S], F32, tag=f"mk{j}", name=f"mk{j}")
                      for j in range(2)]
                for j in range(2):
                    # (a1,v1), (a1,v2), (a2,v1), (a2,v2) for batch half j
                    nc.vector.tensor_mul(mk[j][:, 0:128],
                                         exp_av[j][:, ts(j, 128)], ident[:])
                    nc.vector.tensor_mul(mk[j][:, 128:256],
                                         exp_av[j][:, 256 + 128 * j:384 + 128 * j],
                                         ident[:])
                    nc.vector.tensor_mul(mk[j][:, 256:384],
                                         exp_av[j + 2][:, ts(j, 128)], ident[:])
                    nc.vector.tensor_mul(mk[j][:, 384:512],
                                         exp_av[j + 2][:, 256 + 128 * j:384 + 128 * j],
                                         ident[:])
                num2 = tp.tile([128, 2], F32)
                for j in range(2):
                    nc.vector.reduce_sum(num2[:, j:j + 1], mk[j][:],
                                         axis=mybir.AxisListType.X)
                # ... plus exp'd diagonals of (a1,a2) and (v1,v2)
                mq = tp.tile([128, S], F32, tag="mk0", name="mq")
                for q in range(4):
                    nc.vector.tensor_mul(mq[:, ts(q, 128)],
                                         psum_q[:, ts(q, 128)], ident[:])
                qd = tp.tile([128, 4], F32)
                for q in range(4):
                    nc.vector.reduce_sum(qd[:, q:q + 1], mq[:, ts(q, 128)],
                                         axis=mybir.AxisListType.X)
                eqd = tp.tile([128, 4], F32)
                nc.scalar.activation(eqd[:], qd[:], AF.Exp,
                                     scale=float(GRAM_RCP))
                for j in range(2):
                    nc.vector.tensor_add(num2[:, j:j + 1], num2[:, j:j + 1],
                                         eqd[:, j:j + 1])
                    nc.vector.tensor_add(num2[:, j:j + 1], num2[:, j:j + 1],
                                         eqd[:, j + 2:j + 3])

                # loss = -mean(log(num/den))
                rden = tp.tile([128, 2], F32)
                nc.vector.reciprocal(rden[:], den2[:])
                ratio = tp.tile([128, 2], F32)
                nc.vector.tensor_mul(ratio[:], num2[:], rden[:])
                logr = tp.tile([128, 2], F32R)
                nc.scalar.activation(logr[:], ratio[:], AF.Ln)
                ps_l = psmall.tile([1, 2], F32, tag="rowp", name="ps_l")
                nc.tensor.matmul(ps_l[:], ones_r[:], logr[:],
                                 start=True, stop=True)
                lsum = tp.tile([1, 1], F32)
                nc.vector.tensor_add(lsum[:], ps_l[0:1, 0:1], ps_l[0:1, 1:2])
                loss_sb = tp.tile([1, 1], F32)
                nc.scalar.activation(loss_sb[:], lsum[:], AF.Copy,
                                     scale=float(-1.0 / B))
                nc.sync.dma_start(out=loss_d.ap(), in_=loss_sb[:])

    nc.compile()
    return nc


def _get_nc():
    if "nc" not in _CACHE:
        _CACHE["nc"] = build()
    return _CACHE["nc"]


def _pack_kmajor(m, nkt, width):
    """[K, width] k-major -> [nkt, 128, 2, width] fp8 DoubleRow layout."""
    out = m.reshape(nkt, 2, 128, width).transpose(0, 2, 1, 3)
    return np.ascontiguousarray(out).astype(NP_F8)


def _shard_inputs(a_1, v_1, a_2, v_2, W_a, W_v):
    # audio: (2b,1,80,16) -> (512, 1280); replicated on every core
    A = np.concatenate([a_1, a_2], axis=0).reshape(S, KA)
    xa = _pack_kmajor(np.ascontiguousarray(A.T), NKA, S)
    wa = _pack_kmajor(W_a * np.float32(SC_A), NKA, D)
    # visual: lower half, flattened in (c,t,r,w) order; W_v rows permuted
    # from the reference's (t,c) order to match.
    V = np.concatenate([v_1, v_2], axis=0)
    V = V.reshape(S, 15, 96, 96)[:, :, 48:, :].reshape(S, KV_TOT)
    Vt = np.ascontiguousarray(V.T)
    Wvp = np.ascontiguousarray(
        W_v.reshape(5, 3, 48 * 96, D).transpose(1, 0, 2, 3)
    ).reshape(KV_TOT, D) * np.float32(SC_V)

    in_maps = []
    for c in range(N_CORES):
        vt_p = np.zeros((KP, S), np.float32)
        vt_p[:KV] = Vt[c * KV:(c + 1) * KV]
        xv = _pack_kmajor(vt_p, NKT, S)
        wv_p = np.zeros((KP, D), np.float32)
        wv_p[:KV] = Wvp[c * KV:(c + 1) * KV]
        wv4 = wv_p.reshape(NKT, 2, 128, 2, 256).transpose(3, 0, 2, 1, 4)
        wv = np.ascontiguousarray(wv4).astype(NP_F8)
        in_maps.append({"xv": xv, "wv": wv, "xa": xa, "wa": wa})
    return in_maps


def kernel(a_1, v_1, a_2, v_2, W_a, W_v):
    nc = _get_nc()
    in_maps = _shard_inputs(np.asarray(a_1, np.float32),
                            np.asarray(v_1, np.float32),
                            np.asarray(a_2, np.float32),
                            np.asarray(v_2, np.float32),
                            np.asarray(W_a, np.float32),
                            np.asarray(W_v, np.float32))
    res = bass_utils.run_bass_kernel_spmd(nc, in_maps,
                                          core_ids=list(range(N_CORES)))
    return np.asarray(res.results[0]["loss"], np.float32).reshape(())
